# revision 16
# baseline (speedup 1.0000x reference)
"""Bayesian linear layer (per-sample weights) on 8 Trainium2 NeuronCores.

out[b,o] = sum_i x[b,i] * (eps[b,i,o]*softplus(ro)[i,o] + mu[i,o])
           + eps_bias[b,o]*softplus(ro_bias)[o] + mu_bias[o]

Strategy (2D sharding: 4 batch-groups x 2 output-halves per core):
  - Each core handles 32 samples x 512 output columns, so the replicated
    ro/mu parameter traffic halves (the eps shard is 64MB either way and
    HBM-domain bandwidth, shared by NC pairs, is the binding resource).
  - eps rows for one sample ([1024, 512] f32, 2MB contiguous-strided) are
    DMAed as one [128, 4096] tile (i-chunk on partitions, 8 chunks x 512
    o on free dim) on the sync HWDGE ring; params + small traffic ride
    the scalar ring and the gpsimd (SWDGE) ring so the eps stream is
    never interrupted.
  - DVE multiplies tiles by softplus(ro), rounding to float32r so
    TensorE consumes them at full (1 cycle/row) rate.
  - TensorE reduces over i with M=1 matmuls (lhsT = x[b, chunk] column)
    into a [1,512] PSUM tile per sample; a one-hot K=32 matmul folds in
    the precomputed bias row (x@mu + eps_bias*softplus(ro_bias) +
    mu_bias), the scalar engine copies PSUM -> SBUF, gpsimd stores it.
"""

import numpy as np

import concourse.bass as bass
import concourse.bacc as bacc
import concourse.mybir as mybir
from concourse.masks import make_identity
from concourse.tile import TileContext
from concourse.bass_utils import run_bass_kernel_spmd

F32 = mybir.dt.float32
F32R = mybir.dt.float32r
AF = mybir.ActivationFunctionType

B, IN, OUT = 128, 1024, 1024
NCORES = 8
BG = 4                    # batch groups
OSH = NCORES // BG        # output shards (2)
BS = B // BG              # 32 samples per core
OUTS = OUT // OSH         # 512 output columns per core
P = 128
NCH = IN // P             # 8 i-chunks
FREE = NCH * OUTS         # 4096 free elems per eps tile (one sample)
NH = OUTS // 512          # matmul halves per sample (1)


def build_nc():
    nc = bacc.Bacc(None, target_bir_lowering=False)

    eps_d = nc.declare_dram_parameter("eps", [BS, IN, OUTS], F32, isOutput=False)
    ro_d = nc.declare_dram_parameter("ro", [IN, OUTS], F32, isOutput=False)
    mu_d = nc.declare_dram_parameter("mu", [IN, OUTS], F32, isOutput=False)
    # xt[p, c*BS + b] = x[b, c*128 + p]  (host-side layout transform)
    xt_d = nc.declare_dram_parameter("xt", [P, NCH * BS], F32, isOutput=False)
    eb_d = nc.declare_dram_parameter("eps_bias", [BS, OUTS], F32, isOutput=False)
    rb_d = nc.declare_dram_parameter("ro_bias", [BS, OUTS], F32, isOutput=False)
    mb_d = nc.declare_dram_parameter("mu_bias", [BS, OUTS], F32, isOutput=False)
    out_d = nc.declare_dram_parameter("out", [BS, OUTS], F32, isOutput=True)

    ro_r = ro_d.rearrange("(c p) o -> p c o", p=P)
    mu_r = mu_d.rearrange("(c p) o -> p c o", p=P)

    with TileContext(nc) as tc:
        with (
            tc.tile_pool(name="const", bufs=1) as cpool,
            tc.tile_pool(name="eps", bufs=4) as epool,
            tc.tile_pool(name="mu", bufs=1) as mupool,
            tc.tile_pool(name="epr", bufs=3) as eprpool,
            tc.tile_pool(name="small", bufs=2) as spool,
            tc.tile_pool(name="psmu", bufs=1, space="PSUM") as pmupool,
            tc.tile_pool(name="psum", bufs=4, space="PSUM") as ppool,
        ):
            # ---- softplus(ro): first on the eps ring so it leads --------
            sig = cpool.tile([P, FREE], F32)
            for h in range(2):
                sl = sig[:, h * (FREE // 2) : (h + 1) * (FREE // 2)]
                nc.sync.dma_start(
                    out=sl, in_=ro_r[:, h * (NCH // 2) : (h + 1) * (NCH // 2), :]
                )
                nc.scalar.activation(sl, sl, AF.Exp)
                nc.scalar.activation(sl, sl, AF.Ln, bias=1.0)

            xt = cpool.tile([P, NCH * BS], F32)
            nc.gpsimd.dma_start(out=xt, in_=xt_d[:, :])
            xtr = cpool.tile([P, NCH * BS], F32R)
            nc.vector.tensor_copy(out=xtr, in_=xt)

            ident = cpool.tile([BS, BS], F32)
            make_identity(nc, ident)
            idr = cpool.tile([BS, BS], F32R)
            nc.vector.tensor_copy(out=idr, in_=ident)

            # ---- x @ mu (mu on the scalar ring) -------------------------
            psmu = pmupool.tile([BS, OUTS], F32)
            mt = mupool.tile([P, FREE], F32)
            nc.scalar.dma_start(out=mt, in_=mu_r[:, :, :])
            for c in range(NCH):
                nc.tensor.matmul(
                    psmu[:, :],
                    xt[:, c * BS : (c + 1) * BS],
                    mt[:, c * OUTS : (c + 1) * OUTS],
                    start=(c == 0),
                    stop=(c == NCH - 1),
                )

            # ---- bias row: eps_bias*softplus(ro_bias) + mu_bias + x@mu --
            eb16 = cpool.tile([BS, OUTS], F32)
            nc.gpsimd.dma_start(out=eb16, in_=eb_d[:, :])
            rb16 = cpool.tile([BS, OUTS], F32)
            nc.gpsimd.dma_start(out=rb16, in_=rb_d[:, :])
            mb16 = cpool.tile([BS, OUTS], F32)
            nc.gpsimd.dma_start(out=mb16, in_=mb_d[:, :])
            nc.scalar.activation(rb16, rb16, AF.Exp)
            nc.scalar.activation(rb16, rb16, AF.Ln, bias=1.0)

            nc.vector.tensor_mul(out=eb16, in0=eb16, in1=rb16)
            nc.vector.tensor_add(out=eb16, in0=eb16, in1=mb16)
            b16r = cpool.tile([BS, OUTS], F32R)
            nc.vector.tensor_add(out=b16r, in0=eb16, in1=psmu)

            # ---- main streaming loop ------------------------------------
            for b in range(BS):
                ps = ppool.tile([1, OUTS], F32)
                ep = epool.tile([P, FREE], F32)
                nc.sync.dma_start(
                    out=ep, in_=eps_d[b, :, :].rearrange("(c p) o -> p c o", p=P)
                )
                for q in range(2):
                    epr = eprpool.tile([P, FREE // 2], F32R)
                    nc.vector.tensor_mul(
                        out=epr,
                        in0=ep[:, q * (FREE // 2) : (q + 1) * (FREE // 2)],
                        in1=sig[:, q * (FREE // 2) : (q + 1) * (FREE // 2)],
                    )
                    for c4 in range(NCH // 2):
                        c = (NCH // 2) * q + c4
                        nc.tensor.matmul(
                            ps[0:1, :],
                            xtr[:, c * BS + b : c * BS + b + 1],
                            epr[:, c4 * OUTS : (c4 + 1) * OUTS],
                            start=(q == 0 and c4 == 0),
                            stop=False,
                        )
                # one-hot matmul adds bias row b into the partition-0 PSUM row
                nc.tensor.matmul(
                    ps[0:1, :],
                    idr[:, b : b + 1],
                    b16r[:, :],
                    start=False,
                    stop=True,
                )
                orow = spool.tile([1, OUTS], F32)
                nc.scalar.copy(orow, ps[0:1, :])
                nc.gpsimd.dma_start(out=out_d[b : b + 1, :], in_=orow)

    nc.finalize()
    return nc


_NC_CACHE = None


def _get_nc():
    global _NC_CACHE
    if _NC_CACHE is None:
        _NC_CACHE = build_nc()
    return _NC_CACHE


def kernel(x, mu, ro, mu_bias, ro_bias, eps, eps_bias, _trace=False, _tmpdir=None):
    x = np.ascontiguousarray(np.asarray(x, dtype=np.float32))
    mu = np.ascontiguousarray(np.asarray(mu, dtype=np.float32))
    ro = np.ascontiguousarray(np.asarray(ro, dtype=np.float32))
    mu_bias = np.asarray(mu_bias, dtype=np.float32).reshape(1, OUT)
    ro_bias = np.asarray(ro_bias, dtype=np.float32).reshape(1, OUT)
    eps = np.asarray(eps, dtype=np.float32)
    eps_bias = np.ascontiguousarray(np.asarray(eps_bias, dtype=np.float32))

    nc = _get_nc()

    in_maps = []
    for core in range(NCORES):
        g, j = core // OSH, core % OSH
        b0, b1 = g * BS, (g + 1) * BS
        o0, o1 = j * OUTS, (j + 1) * OUTS
        x_sh = x[b0:b1]  # (BS, IN)
        # xt[p, c*BS + b] = x_sh[b, c*128 + p]
        xt = np.ascontiguousarray(
            x_sh.reshape(BS, NCH, P).transpose(2, 1, 0).reshape(P, NCH * BS)
        )
        in_maps.append(
            {
                "eps": np.ascontiguousarray(eps[b0:b1, :, o0:o1]),
                "ro": np.ascontiguousarray(ro[:, o0:o1]),
                "mu": np.ascontiguousarray(mu[:, o0:o1]),
                "xt": xt,
                "eps_bias": np.ascontiguousarray(eps_bias[b0:b1, o0:o1]),
                "ro_bias": np.ascontiguousarray(
                    np.broadcast_to(ro_bias[:, o0:o1], (BS, OUTS))
                ),
                "mu_bias": np.ascontiguousarray(
                    np.broadcast_to(mu_bias[:, o0:o1], (BS, OUTS))
                ),
            }
        )

    res = run_bass_kernel_spmd(
        nc, in_maps, core_ids=list(range(NCORES)), trace=_trace, tmpdir=_tmpdir
    )
    out = np.empty((B, OUT), dtype=np.float32)
    for core in range(NCORES):
        g, j = core // OSH, core % OSH
        out[g * BS : (g + 1) * BS, j * OUTS : (j + 1) * OUTS] = res.results[core]["out"]
    if _trace:
        kernel.last_results = res
    return out
